# revision 20
# baseline (speedup 1.0000x reference)
"""Trainium2 Bass kernel for nn_BERTEmbedding (fused per-index affine + sinusoidal PE).

Math (per batch b, vocab-position v, embed index e):
    out[b,v,e] = s0[b,v]*flux_w[v,e] + flux_b[v,e]
               + s2[b,v]*time_w[v,e] + time_b[v,e]
               + (e even: sin(s1[b,v]*div[e/2]) ; e odd: cos(s1[b,v]*div[(e-1)/2]))

Sharding: vocab axis V=4096 split across 8 cores (512 rows each); every core
handles all 16 batches of its vocab shard.  Weight tables are sharded with the
vocab axis and shipped in bf16 (halves table DMA; ~0.2% relative error on the
small affine terms, far under the 2e-2 gate).

Device strategy (per core, 4 v-tiles x 16 batches = 64 items of [128,768]):
  - TensorE: psum = diag(s0) @ fw + diag(s2) @ tw + I @ bsum   (bf16 matmuls;
    bsum = flux_b + time_b folded on host)
  - ScalarE: sin/cos evals batched 8 batches per ACTIVATE, plus half the
    angle staging (Copy with per-partition scale=s1).  Sin valid on [-pi,pi]:
      k >= KLO: |s1|*d_KLO + pi/2 < pi for this problem -> direct Sin
      k <  KLO: host ships fully wrapped+clipped angles in fp16 (alo)
  - GpSimd: other half of angle staging (f32 tensor_tensor broadcast) and the
    diag builds -- one all-bf16 TT per pair building all 4 diags at once.
  - VectorE: ONLY the psum+pe merges (TT, 1x, never port-contends), batched
    2 items per op; interleaves sin/cos via the read access pattern.
  - DMA: all loads prefetched up front; stores batched 2 items (786KB).

Software pipelining: the angle staging ops for v-tile vt+1 are dripped into
v-tile vt's pair loop (2 per pair) so sins of vt+1 start immediately at the
v-tile boundary; a dummy 1-element Sin right after the memsets pulls the
~2.7us ACT table load into the DMA prefetch window.

Engine budget (predicted, warm): ACT ~65us, GpSimd ~61, DVE ~60, PE ~62,
DMA wire ~82us (29.4MB @ ~358GB/s) -> target ~90us.
"""

import math

import numpy as np

try:
    import concourse.bass as bass
except ImportError:  # harness containers keep the repo at /opt/trn_rl_repo
    import sys

    sys.path.insert(0, "/opt/trn_rl_repo")
    import concourse.bass as bass

import concourse.bacc as bacc
import concourse.tile as tile
from concourse import mybir
from concourse.bass_utils import run_bass_kernel_spmd

B, V, E = 16, 4096, 768
EH = E // 2  # 384 angle lanes
KLO = 48  # angle lanes shipped pre-wrapped from host
KHI = EH - KLO  # 336 direct-Sin lanes
N_CORES = 8
V_SHARD = V // N_CORES  # 512
VT = V_SHARD // 128  # 4 v-tiles per core
GA = 16  # batches per ACT sin op
F32 = mybir.dt.float32
BF16 = mybir.dt.bfloat16
FP16 = mybir.dt.float16

TWO_PI = 2.0 * math.pi
HALF_PI = float(np.float32(math.pi / 2.0))
# keep shipped lo angles strictly inside ScalarE's [-pi, pi] spline domain
ALO_CLIP = math.pi - 2e-3
# direct-Sin lanes need |s1|*d_KLO + pi/2 <= pi
S1_LIMIT = (math.pi / 2.0) / math.exp(-KLO * math.log(10000.0) / EH)


def build_bass() -> "bass.Bass":
    from contextlib import ExitStack

    nc = bacc.Bacc(
        "TRN2",
        target_bir_lowering=False,
        debug=False,
        num_devices=N_CORES,
    )
    Alu = mybir.AluOpType

    # consolidated input blobs (one DMA each, ordered for startup latency)
    f32b_d = nc.dram_tensor("f32b", [128, VT * B + KHI], F32, kind="ExternalInput")
    bf0_d = nc.dram_tensor("bf0", [128, 3 * E + VT * B * 2 + 128], BF16,
                           kind="ExternalInput")
    alo0_d = nc.dram_tensor("alo0", [128, B * 2 * KLO], FP16, kind="ExternalInput")
    bfr_d = nc.dram_tensor("bfr", [128, (VT - 1) * 3 * E], BF16,
                           kind="ExternalInput")
    alor_d = nc.dram_tensor("alor", [128, (VT - 1) * B * 2 * KLO], FP16,
                            kind="ExternalInput")
    out_d = nc.dram_tensor("out", [B, V_SHARD, E], F32, kind="ExternalOutput")

    with tile.TileContext(nc) as tc, ExitStack() as ctx:
        const_pool = ctx.enter_context(tc.tile_pool(name="const", bufs=1))
        ang_pool = ctx.enter_context(tc.tile_pool(name="ang", bufs=2))
        pe_pool = ctx.enter_context(tc.tile_pool(name="pe", bufs=2))
        diag_pool = ctx.enter_context(tc.tile_pool(name="diag", bufs=6))
        out_pool = ctx.enter_context(tc.tile_pool(name="out", bufs=6))
        psum_pool = ctx.enter_context(tc.tile_pool(name="psum", bufs=2, space="PSUM"))

        zero_t = const_pool.tile([128, 1], F32, tag="zero")
        nc.vector.memset(zero_t[:], 0.0)
        hpi_t = const_pool.tile([128, 1], F32, tag="hpi")
        nc.vector.memset(hpi_t[:], HALF_PI)
        # dummy Sin to pull the ACT table load into the prefetch window
        scratch_t = const_pool.tile([128, 1], F32, tag="scratch")
        nc.scalar.activation(
            scratch_t[:],
            zero_t[:],
            mybir.ActivationFunctionType.Sin,
            bias=zero_t[:],
            scale=1.0,
        )

        # prefetch everything in 5 blob DMAs, ordered for startup latency:
        # f32 staging consts -> vt0 tables -> vt0 lo angles -> the rest
        f32b_t = const_pool.tile([128, VT * B + KHI], F32, tag="f32b")
        nc.sync.dma_start(f32b_t[:], f32b_d[:])
        bf0_t = const_pool.tile([128, 3 * E + VT * B * 2 + 128], BF16, tag="bf0")
        nc.sync.dma_start(bf0_t[:], bf0_d[:])
        alo0_t = const_pool.tile([128, B * 2 * KLO], FP16, tag="alo0")
        nc.sync.dma_start(alo0_t[:], alo0_d[:])
        bfr_t = const_pool.tile([128, (VT - 1) * 3 * E], BF16, tag="bfr")
        nc.sync.dma_start(bfr_t[:], bfr_d[:])
        alor_t = const_pool.tile([128, (VT - 1) * B * 2 * KLO], FP16, tag="alor")
        nc.sync.dma_start(alor_t[:], alor_d[:])

        s1c_t = f32b_t[:, 0 : VT * B]
        dv_t = f32b_t[:, VT * B : VT * B + KHI]
        sc_t = bf0_t[:, 3 * E : 3 * E + VT * B * 2]
        eye_t = bf0_t[:, 3 * E + VT * B * 2 : 3 * E + VT * B * 2 + 128]

        fw_ts, tw_ts, bs_ts, alo_ts = [], [], [], []
        for vt in range(VT):
            if vt == 0:
                base_t, base = bf0_t, 0
                alo_ts.append(alo0_t[:])
            else:
                base_t, base = bfr_t, (vt - 1) * 3 * E
                alo_ts.append(
                    alor_t[:, (vt - 1) * B * 2 * KLO : vt * B * 2 * KLO]
                )
            fw_ts.append(base_t[:, base : base + E])
            tw_ts.append(base_t[:, base + E : base + 2 * E])
            bs_ts.append(base_t[:, base + 2 * E : base + 3 * E])

        def emit_ang(ang, vt, b, eng=None):
            """Stage hi angles for one batch: ang[:, b*KHI:...] = s1 * dv.
            By default batches 0..7 go to GpSimd, 8..15 to ScalarE so the
            two halves run concurrently."""
            s1 = s1c_t[:, vt * B + b : vt * B + b + 1]
            dst = ang[:, b * KHI : (b + 1) * KHI]
            if eng == "g" or (eng is None and b < 10):
                nc.gpsimd.tensor_tensor(
                    dst, dv_t, s1.broadcast_to((128, KHI)), Alu.mult
                )
            else:
                nc.scalar.mul(dst, dv_t, s1)

        def emit_sins(pe_i, ang_i, alo_t, b0, nb):
            """sin/cos for batches [b0, b0+nb) of the current v-tile."""
            bsl = slice(b0, b0 + nb)
            nc.scalar.activation(
                pe_i[:, bsl, KLO:EH],
                ang_i[:, bsl, :],
                mybir.ActivationFunctionType.Sin,
                bias=zero_t[:],
                scale=1.0,
            )
            nc.scalar.activation(
                pe_i[:, bsl, EH + KLO : E],
                ang_i[:, bsl, :],
                mybir.ActivationFunctionType.Sin,
                bias=hpi_t[:],
                scale=1.0,
            )
            nc.scalar.activation(
                pe_i.rearrange("p i (h q) -> p i h q", h=2)[:, bsl, :, 0:KLO],
                alo_t.rearrange("p (i h q) -> p i h q", i=B, h=2)[:, bsl, :, :],
                mybir.ActivationFunctionType.Sin,
                bias=zero_t[:],
                scale=1.0,
            )

        ang_ts = [None] * VT
        ang_ts[0] = ang_pool.tile([128, B * KHI], F32, tag="ang", name="ang0")

        for vt in range(VT):
            fw_t, tw_t, bs_t, alo_t = fw_ts[vt], tw_ts[vt], bs_ts[vt], alo_ts[vt]
            ang = ang_ts[vt]

            pe = pe_pool.tile([128, B * E], BF16, tag="pe")
            pe_i = pe[:].rearrange("p (i e) -> p i e", i=B)
            ang_i = ang[:].rearrange("p (i k) -> p i k", i=B)
            if vt > 0:
                # angles were dripped in during the previous v-tile
                for g in range(B // GA):
                    emit_sins(pe_i, ang_i, alo_t, g * GA, GA)

            if vt + 1 < VT:
                ang_ts[vt + 1] = ang_pool.tile(
                    [128, B * KHI], F32, tag="ang", name=f"ang{vt + 1}"
                )

            for pair in range(B // 2):
                if vt == 0:
                    # fine-grained startup: stage+sin 2 batches, then compute
                    # their pair immediately so stores start while the input
                    # prefetch is still streaming
                    emit_ang(ang, 0, 2 * pair, eng="g")
                    emit_ang(ang, 0, 2 * pair + 1, eng="g")
                    emit_sins(pe_i, ang_i, alo_t, 2 * pair, 2)
                b0 = pair * 2
                scc = (vt * B + b0) * 2

                # all 4 diags (d0,d2 for both items) in one all-bf16 TT
                dall = diag_pool.tile([128, 4 * 128], BF16, tag="dall")
                nc.gpsimd.tensor_tensor(
                    dall[:].rearrange("p (c x) -> p c x", c=4),
                    eye_t.rearrange("p (u x) -> p u x", u=1).broadcast_to(
                        (128, 4, 128)
                    ),
                    sc_t[:, scc : scc + 4]
                    .rearrange("p (c u) -> p c u", u=1)
                    .broadcast_to((128, 4, 128)),
                    Alu.mult,
                )

                ps = psum_pool.tile([128, 2048], F32, tag="ps")
                for j in (0, 1):
                    # psum[:, j*1024 : j*1024+768] = d0@fw + d2@tw + I@bs
                    # (512/256 split keeps each matmul inside one PSUM bank)
                    off = j * 1024
                    d0 = dall[:, (2 * j) * 128 : (2 * j + 1) * 128]
                    d2 = dall[:, (2 * j + 1) * 128 : (2 * j + 2) * 128]
                    for w, t in ((d0, fw_t), (d2, tw_t), (eye_t, bs_t)):
                        for lo, hi in ((0, 512), (512, E)):
                            nc.tensor.matmul(
                                ps[:, off + lo : off + hi],
                                w,
                                t[:, lo:hi],
                                start=t is fw_t,
                                stop=t is bs_t,
                            )

                # merge both items; interleave sin/cos via the read pattern
                out2 = out_pool.tile([128, 2 * E], F32, tag="o")
                nc.vector.tensor_add(
                    out2[:].rearrange("p (j q h) -> p j q h", j=2, h=2),
                    ps[:]
                    .rearrange("p (j x) -> p j x", j=2)[:, :, 0:E]
                    .rearrange("p j (q h) -> p j q h", h=2),
                    pe[:, b0 * E : (b0 + 2) * E].rearrange(
                        "p (j h q) -> p j q h", j=2, h=2
                    ),
                )

                nc.sync.dma_start(
                    out_d[b0 : b0 + 2, vt * 128 : (vt + 1) * 128, :].rearrange(
                        "j p e -> p j e"
                    ),
                    out2[:].rearrange("p (j e) -> p j e", j=2),
                )

                # drip next v-tile's angle staging into this pair loop
                if vt + 1 < VT:
                    emit_ang(ang_ts[vt + 1], vt + 1, 2 * pair)
                    emit_ang(ang_ts[vt + 1], vt + 1, 2 * pair + 1)

    nc.finalize()
    return nc


_NC_CACHE: list = []


def _get_nc():
    if not _NC_CACHE:
        _NC_CACHE.append(build_bass())
    return _NC_CACHE[0]


def make_in_maps(sequence, flux_w, flux_b, time_w, time_b):
    import ml_dtypes

    sequence = np.asarray(sequence, dtype=np.float32)
    bsum = np.asarray(flux_b, dtype=np.float32) + np.asarray(time_b, dtype=np.float32)
    fw_bf = np.asarray(flux_w, dtype=np.float32).astype(ml_dtypes.bfloat16)
    tw_bf = np.asarray(time_w, dtype=np.float32).astype(ml_dtypes.bfloat16)
    bs_bf = bsum.astype(ml_dtypes.bfloat16)

    s1_all = sequence[:, :, 1]
    assert np.abs(s1_all).max() < S1_LIMIT, (
        f"positional channel exceeds direct-Sin range: {np.abs(s1_all).max():.3f} "
        f">= {S1_LIMIT:.3f}; raise KLO"
    )

    div = np.exp(
        np.arange(0, E, 2, dtype=np.float32) * np.float32(-math.log(10000.0) / E)
    ).astype(np.float32)
    dv_rep = np.ascontiguousarray(np.broadcast_to(div[KLO:], (128, KHI)))
    eye = np.eye(128, dtype=np.float32).astype(ml_dtypes.bfloat16)

    # lo lanes: fully wrapped angles, fp16.  alo[b,v,h*KLO+k] =
    # wrap(s1*d_k + h*pi/2) into (-pi, pi), clipped inside the spline domain.
    jj = np.concatenate([np.zeros(KLO, np.float64), np.ones(KLO, np.float64)])
    dd = np.concatenate([div[:KLO], div[:KLO]]).astype(np.float64)
    ang = s1_all[:, :, None].astype(np.float64) * dd[None, None, :] + jj * (
        math.pi / 2.0
    )
    wrapped = ang - TWO_PI * np.rint(ang / TWO_PI)
    alo = np.clip(wrapped, -ALO_CLIP, ALO_CLIP).astype(np.float16)

    in_maps = []
    for c in range(N_CORES):
        v0, v1 = c * V_SHARD, (c + 1) * V_SHARD
        s = sequence[:, v0:v1, :].reshape(B, VT, 128, 3)
        s_r = s.transpose(2, 1, 0, 3)  # [128, VT, B, 3]
        # (s0, s2) pairs in bf16: [128p, (vt*B + b)*2 + {0,1}]
        sc_r = (
            np.ascontiguousarray(s_r[:, :, :, 0::2])
            .reshape(128, VT * B * 2)
            .astype(ml_dtypes.bfloat16)
        )
        # s1 in f32: [128p, vt*B + b]
        s1c_r = np.ascontiguousarray(s_r[:, :, :, 1]).reshape(128, VT * B)
        # alo [B, 512, 2*KLO] -> [128p, (vt*B + b)*2*KLO + lane]
        a = alo[:, v0:v1, :].reshape(B, VT, 128, 2 * KLO)
        alo_r = np.ascontiguousarray(a.transpose(2, 1, 0, 3)).reshape(
            128, VT * B * 2 * KLO
        )
        # per-v-tile tables with v-row on the partition axis: [VT, 128, E]
        fw_v = fw_bf[v0:v1].reshape(VT, 128, E)
        tw_v = tw_bf[v0:v1].reshape(VT, 128, E)
        bs_v = bs_bf[v0:v1].reshape(VT, 128, E)

        f32b = np.concatenate([s1c_r, dv_rep], axis=1)
        bf0 = np.concatenate([fw_v[0], tw_v[0], bs_v[0], sc_r, eye], axis=1)
        bfr = np.concatenate(
            sum([[fw_v[t], tw_v[t], bs_v[t]] for t in range(1, VT)], []), axis=1
        )
        in_maps.append(
            {
                "f32b": np.ascontiguousarray(f32b),
                "bf0": np.ascontiguousarray(bf0),
                "alo0": np.ascontiguousarray(alo_r[:, : B * 2 * KLO]),
                "bfr": np.ascontiguousarray(bfr),
                "alor": np.ascontiguousarray(alo_r[:, B * 2 * KLO :]),
            }
        )
    return in_maps


def run(in_maps, trace: bool = False):
    nc = _get_nc()
    return run_bass_kernel_spmd(nc, in_maps, list(range(N_CORES)), trace=trace)


def kernel(sequence, flux_w, flux_b, time_w, time_b) -> np.ndarray:
    in_maps = make_in_maps(sequence, flux_w, flux_b, time_w, time_b)
    res = run(in_maps)
    out = np.concatenate([res.results[c]["out"] for c in range(N_CORES)], axis=1)
    return np.ascontiguousarray(out.astype(np.float32, copy=False))


# revision 21
# speedup vs baseline: 1.1008x; 1.1008x over previous
"""Trainium2 Bass kernel for nn_BERTEmbedding (fused per-index affine + sinusoidal PE).

Math (per batch b, vocab-position v, embed index e):
    out[b,v,e] = s0[b,v]*flux_w[v,e] + flux_b[v,e]
               + s2[b,v]*time_w[v,e] + time_b[v,e]
               + (e even: sin(s1[b,v]*div[e/2]) ; e odd: cos(s1[b,v]*div[(e-1)/2]))

Sharding: vocab axis V=4096 split across 8 cores (512 rows each); every core
handles all 16 batches of its vocab shard.  Weight tables are sharded with the
vocab axis and shipped in bf16 (halves table DMA; ~0.2% relative error on the
small affine terms, far under the 2e-2 gate).

Device strategy (per core, 4 v-tiles x 16 batches = 64 items of [128,768]):
  - TensorE: psum = diag(s0) @ fw + diag(s2) @ tw + I @ bsum   (bf16 matmuls;
    bsum = flux_b + time_b folded on host)
  - ScalarE: sin/cos evals batched 8 batches per ACTIVATE, plus half the
    angle staging (Copy with per-partition scale=s1).  Sin valid on [-pi,pi]:
      k >= KLO: |s1|*d_KLO + pi/2 < pi for this problem -> direct Sin
      k <  KLO: host ships fully wrapped+clipped angles in fp16 (alo)
  - GpSimd: other half of angle staging (f32 tensor_tensor broadcast) and the
    diag builds -- one all-bf16 TT per pair building all 4 diags at once.
  - VectorE: ONLY the psum+pe merges (TT, 1x, never port-contends), batched
    2 items per op; interleaves sin/cos via the read access pattern.
  - DMA: all loads prefetched up front; stores batched 2 items (786KB).

Software pipelining: the angle staging ops for v-tile vt+1 are dripped into
v-tile vt's pair loop (2 per pair) so sins of vt+1 start immediately at the
v-tile boundary; a dummy 1-element Sin right after the memsets pulls the
~2.7us ACT table load into the DMA prefetch window.

Engine budget (predicted, warm): ACT ~65us, GpSimd ~61, DVE ~60, PE ~62,
DMA wire ~82us (29.4MB @ ~358GB/s) -> target ~90us.
"""

import math

import numpy as np

try:
    import concourse.bass as bass
except ImportError:  # harness containers keep the repo at /opt/trn_rl_repo
    import sys

    sys.path.insert(0, "/opt/trn_rl_repo")
    import concourse.bass as bass

import concourse.bacc as bacc
import concourse.tile as tile
from concourse import mybir
from concourse.bass_utils import run_bass_kernel_spmd

B, V, E = 16, 4096, 768
EH = E // 2  # 384 angle lanes
KLO = 48  # angle lanes shipped pre-wrapped from host
KHI = EH - KLO  # 336 direct-Sin lanes
N_CORES = 8
V_SHARD = V // N_CORES  # 512
VT = V_SHARD // 128  # 4 v-tiles per core
GA = 8  # batches per ACT sin op
F32 = mybir.dt.float32
BF16 = mybir.dt.bfloat16
FP16 = mybir.dt.float16

TWO_PI = 2.0 * math.pi
HALF_PI = float(np.float32(math.pi / 2.0))
# keep shipped lo angles strictly inside ScalarE's [-pi, pi] spline domain
ALO_CLIP = math.pi - 2e-3
# direct-Sin lanes need |s1|*d_KLO + pi/2 <= pi
S1_LIMIT = (math.pi / 2.0) / math.exp(-KLO * math.log(10000.0) / EH)


def build_bass() -> "bass.Bass":
    from contextlib import ExitStack

    nc = bacc.Bacc(
        "TRN2",
        target_bir_lowering=False,
        debug=False,
        num_devices=N_CORES,
    )
    Alu = mybir.AluOpType

    # consolidated input blobs (one DMA each, ordered for startup latency)
    f32b_d = nc.dram_tensor("f32b", [128, VT * B + KHI], F32, kind="ExternalInput")
    bf0_d = nc.dram_tensor("bf0", [128, 3 * E + VT * B * 2 + 128], BF16,
                           kind="ExternalInput")
    alo0_d = nc.dram_tensor("alo0", [128, B * 2 * KLO], FP16, kind="ExternalInput")
    bfr_d = nc.dram_tensor("bfr", [128, (VT - 1) * 3 * E], BF16,
                           kind="ExternalInput")
    alor_d = nc.dram_tensor("alor", [128, (VT - 1) * B * 2 * KLO], FP16,
                            kind="ExternalInput")
    out_d = nc.dram_tensor("out", [B, V_SHARD, E], F32, kind="ExternalOutput")

    with tile.TileContext(nc) as tc, ExitStack() as ctx:
        const_pool = ctx.enter_context(tc.tile_pool(name="const", bufs=1))
        ang_pool = ctx.enter_context(tc.tile_pool(name="ang", bufs=2))
        pe_pool = ctx.enter_context(tc.tile_pool(name="pe", bufs=2))
        diag_pool = ctx.enter_context(tc.tile_pool(name="diag", bufs=6))
        out_pool = ctx.enter_context(tc.tile_pool(name="out", bufs=6))
        psum_pool = ctx.enter_context(tc.tile_pool(name="psum", bufs=2, space="PSUM"))

        zero_t = const_pool.tile([128, 1], F32, tag="zero")
        nc.vector.memset(zero_t[:], 0.0)
        hpi_t = const_pool.tile([128, 1], F32, tag="hpi")
        nc.vector.memset(hpi_t[:], HALF_PI)
        # dummy Sin to pull the ACT table load into the prefetch window
        scratch_t = const_pool.tile([128, 1], F32, tag="scratch")
        nc.scalar.activation(
            scratch_t[:],
            zero_t[:],
            mybir.ActivationFunctionType.Sin,
            bias=zero_t[:],
            scale=1.0,
        )

        # prefetch everything in 5 blob DMAs, ordered for startup latency:
        # f32 staging consts -> vt0 tables -> vt0 lo angles -> the rest
        f32b_t = const_pool.tile([128, VT * B + KHI], F32, tag="f32b")
        nc.sync.dma_start(f32b_t[:], f32b_d[:])
        bf0_t = const_pool.tile([128, 3 * E + VT * B * 2 + 128], BF16, tag="bf0")
        nc.sync.dma_start(bf0_t[:], bf0_d[:])
        alo0_t = const_pool.tile([128, B * 2 * KLO], FP16, tag="alo0")
        nc.sync.dma_start(alo0_t[:], alo0_d[:])
        bfr_t = const_pool.tile([128, (VT - 1) * 3 * E], BF16, tag="bfr")
        nc.sync.dma_start(bfr_t[:], bfr_d[:])
        alor_t = const_pool.tile([128, (VT - 1) * B * 2 * KLO], FP16, tag="alor")
        nc.sync.dma_start(alor_t[:], alor_d[:])

        s1c_t = f32b_t[:, 0 : VT * B]
        dv_t = f32b_t[:, VT * B : VT * B + KHI]
        sc_t = bf0_t[:, 3 * E : 3 * E + VT * B * 2]
        eye_t = bf0_t[:, 3 * E + VT * B * 2 : 3 * E + VT * B * 2 + 128]

        fw_ts, tw_ts, bs_ts, alo_ts = [], [], [], []
        for vt in range(VT):
            if vt == 0:
                base_t, base = bf0_t, 0
                alo_ts.append(alo0_t[:])
            else:
                base_t, base = bfr_t, (vt - 1) * 3 * E
                alo_ts.append(
                    alor_t[:, (vt - 1) * B * 2 * KLO : vt * B * 2 * KLO]
                )
            fw_ts.append(base_t[:, base : base + E])
            tw_ts.append(base_t[:, base + E : base + 2 * E])
            bs_ts.append(base_t[:, base + 2 * E : base + 3 * E])

        def emit_ang(ang, vt, b, eng=None):
            """Stage hi angles for one batch: ang[:, b*KHI:...] = s1 * dv.
            By default batches 0..7 go to GpSimd, 8..15 to ScalarE so the
            two halves run concurrently."""
            s1 = s1c_t[:, vt * B + b : vt * B + b + 1]
            dst = ang[:, b * KHI : (b + 1) * KHI]
            if eng == "g" or (eng is None and b < 10):
                nc.gpsimd.tensor_tensor(
                    dst, dv_t, s1.broadcast_to((128, KHI)), Alu.mult
                )
            else:
                nc.scalar.mul(dst, dv_t, s1)

        def emit_sins(pe_i, ang_i, alo_t, b0, nb):
            """sin/cos for batches [b0, b0+nb) of the current v-tile."""
            bsl = slice(b0, b0 + nb)
            nc.scalar.activation(
                pe_i[:, bsl, KLO:EH],
                ang_i[:, bsl, :],
                mybir.ActivationFunctionType.Sin,
                bias=zero_t[:],
                scale=1.0,
            )
            nc.scalar.activation(
                pe_i[:, bsl, EH + KLO : E],
                ang_i[:, bsl, :],
                mybir.ActivationFunctionType.Sin,
                bias=hpi_t[:],
                scale=1.0,
            )
            nc.scalar.activation(
                pe_i.rearrange("p i (h q) -> p i h q", h=2)[:, bsl, :, 0:KLO],
                alo_t.rearrange("p (i h q) -> p i h q", i=B, h=2)[:, bsl, :, :],
                mybir.ActivationFunctionType.Sin,
                bias=zero_t[:],
                scale=1.0,
            )

        ang_ts = [None] * VT
        ang_ts[0] = ang_pool.tile([128, B * KHI], F32, tag="ang", name="ang0")

        for vt in range(VT):
            fw_t, tw_t, bs_t, alo_t = fw_ts[vt], tw_ts[vt], bs_ts[vt], alo_ts[vt]
            ang = ang_ts[vt]

            pe = pe_pool.tile([128, B * E], BF16, tag="pe")
            pe_i = pe[:].rearrange("p (i e) -> p i e", i=B)
            ang_i = ang[:].rearrange("p (i k) -> p i k", i=B)
            if vt > 0:
                # angles were dripped in during the previous v-tile
                for g in range(B // GA):
                    emit_sins(pe_i, ang_i, alo_t, g * GA, GA)

            if vt + 1 < VT:
                ang_ts[vt + 1] = ang_pool.tile(
                    [128, B * KHI], F32, tag="ang", name=f"ang{vt + 1}"
                )

            for pair in range(B // 2):
                if vt == 0:
                    # fine-grained startup: stage+sin 2 batches, then compute
                    # their pair immediately so stores start while the input
                    # prefetch is still streaming
                    emit_ang(ang, 0, 2 * pair, eng="g")
                    emit_ang(ang, 0, 2 * pair + 1, eng="g")
                    emit_sins(pe_i, ang_i, alo_t, 2 * pair, 2)
                b0 = pair * 2
                scc = (vt * B + b0) * 2

                # all 4 diags (d0,d2 for both items) in one all-bf16 TT
                dall = diag_pool.tile([128, 4 * 128], BF16, tag="dall")
                nc.gpsimd.tensor_tensor(
                    dall[:].rearrange("p (c x) -> p c x", c=4),
                    eye_t.rearrange("p (u x) -> p u x", u=1).broadcast_to(
                        (128, 4, 128)
                    ),
                    sc_t[:, scc : scc + 4]
                    .rearrange("p (c u) -> p c u", u=1)
                    .broadcast_to((128, 4, 128)),
                    Alu.mult,
                )

                ps = psum_pool.tile([128, 2048], F32, tag="ps")
                for j in (0, 1):
                    # psum[:, j*1024 : j*1024+768] = d0@fw + d2@tw + I@bs
                    # (512/256 split keeps each matmul inside one PSUM bank)
                    off = j * 1024
                    d0 = dall[:, (2 * j) * 128 : (2 * j + 1) * 128]
                    d2 = dall[:, (2 * j + 1) * 128 : (2 * j + 2) * 128]
                    for w, t in ((d0, fw_t), (d2, tw_t), (eye_t, bs_t)):
                        for lo, hi in ((0, 512), (512, E)):
                            nc.tensor.matmul(
                                ps[:, off + lo : off + hi],
                                w,
                                t[:, lo:hi],
                                start=t is fw_t,
                                stop=t is bs_t,
                            )

                # merge both items; interleave sin/cos via the read pattern
                out2 = out_pool.tile([128, 2 * E], F32, tag="o")
                nc.vector.tensor_add(
                    out2[:].rearrange("p (j q h) -> p j q h", j=2, h=2),
                    ps[:]
                    .rearrange("p (j x) -> p j x", j=2)[:, :, 0:E]
                    .rearrange("p j (q h) -> p j q h", h=2),
                    pe[:, b0 * E : (b0 + 2) * E].rearrange(
                        "p (j h q) -> p j q h", j=2, h=2
                    ),
                )

                nc.sync.dma_start(
                    out_d[b0 : b0 + 2, vt * 128 : (vt + 1) * 128, :].rearrange(
                        "j p e -> p j e"
                    ),
                    out2[:].rearrange("p (j e) -> p j e", j=2),
                )

                # drip next v-tile's angle staging into this pair loop
                if vt + 1 < VT:
                    emit_ang(ang_ts[vt + 1], vt + 1, 2 * pair)
                    emit_ang(ang_ts[vt + 1], vt + 1, 2 * pair + 1)

    nc.finalize()
    return nc


_NC_CACHE: list = []


def _get_nc():
    if not _NC_CACHE:
        _NC_CACHE.append(build_bass())
    return _NC_CACHE[0]


def make_in_maps(sequence, flux_w, flux_b, time_w, time_b):
    import ml_dtypes

    sequence = np.asarray(sequence, dtype=np.float32)
    bsum = np.asarray(flux_b, dtype=np.float32) + np.asarray(time_b, dtype=np.float32)
    fw_bf = np.asarray(flux_w, dtype=np.float32).astype(ml_dtypes.bfloat16)
    tw_bf = np.asarray(time_w, dtype=np.float32).astype(ml_dtypes.bfloat16)
    bs_bf = bsum.astype(ml_dtypes.bfloat16)

    s1_all = sequence[:, :, 1]
    assert np.abs(s1_all).max() < S1_LIMIT, (
        f"positional channel exceeds direct-Sin range: {np.abs(s1_all).max():.3f} "
        f">= {S1_LIMIT:.3f}; raise KLO"
    )

    div = np.exp(
        np.arange(0, E, 2, dtype=np.float32) * np.float32(-math.log(10000.0) / E)
    ).astype(np.float32)
    dv_rep = np.ascontiguousarray(np.broadcast_to(div[KLO:], (128, KHI)))
    eye = np.eye(128, dtype=np.float32).astype(ml_dtypes.bfloat16)

    # lo lanes: fully wrapped angles, fp16.  alo[b,v,h*KLO+k] =
    # wrap(s1*d_k + h*pi/2) into (-pi, pi), clipped inside the spline domain.
    jj = np.concatenate([np.zeros(KLO, np.float64), np.ones(KLO, np.float64)])
    dd = np.concatenate([div[:KLO], div[:KLO]]).astype(np.float64)
    ang = s1_all[:, :, None].astype(np.float64) * dd[None, None, :] + jj * (
        math.pi / 2.0
    )
    wrapped = ang - TWO_PI * np.rint(ang / TWO_PI)
    alo = np.clip(wrapped, -ALO_CLIP, ALO_CLIP).astype(np.float16)

    in_maps = []
    for c in range(N_CORES):
        v0, v1 = c * V_SHARD, (c + 1) * V_SHARD
        s = sequence[:, v0:v1, :].reshape(B, VT, 128, 3)
        s_r = s.transpose(2, 1, 0, 3)  # [128, VT, B, 3]
        # (s0, s2) pairs in bf16: [128p, (vt*B + b)*2 + {0,1}]
        sc_r = (
            np.ascontiguousarray(s_r[:, :, :, 0::2])
            .reshape(128, VT * B * 2)
            .astype(ml_dtypes.bfloat16)
        )
        # s1 in f32: [128p, vt*B + b]
        s1c_r = np.ascontiguousarray(s_r[:, :, :, 1]).reshape(128, VT * B)
        # alo [B, 512, 2*KLO] -> [128p, (vt*B + b)*2*KLO + lane]
        a = alo[:, v0:v1, :].reshape(B, VT, 128, 2 * KLO)
        alo_r = np.ascontiguousarray(a.transpose(2, 1, 0, 3)).reshape(
            128, VT * B * 2 * KLO
        )
        # per-v-tile tables with v-row on the partition axis: [VT, 128, E]
        fw_v = fw_bf[v0:v1].reshape(VT, 128, E)
        tw_v = tw_bf[v0:v1].reshape(VT, 128, E)
        bs_v = bs_bf[v0:v1].reshape(VT, 128, E)

        f32b = np.concatenate([s1c_r, dv_rep], axis=1)
        bf0 = np.concatenate([fw_v[0], tw_v[0], bs_v[0], sc_r, eye], axis=1)
        bfr = np.concatenate(
            sum([[fw_v[t], tw_v[t], bs_v[t]] for t in range(1, VT)], []), axis=1
        )
        in_maps.append(
            {
                "f32b": np.ascontiguousarray(f32b),
                "bf0": np.ascontiguousarray(bf0),
                "alo0": np.ascontiguousarray(alo_r[:, : B * 2 * KLO]),
                "bfr": np.ascontiguousarray(bfr),
                "alor": np.ascontiguousarray(alo_r[:, B * 2 * KLO :]),
            }
        )
    return in_maps


def run(in_maps, trace: bool = False):
    nc = _get_nc()
    return run_bass_kernel_spmd(nc, in_maps, list(range(N_CORES)), trace=trace)


def kernel(sequence, flux_w, flux_b, time_w, time_b) -> np.ndarray:
    in_maps = make_in_maps(sequence, flux_w, flux_b, time_w, time_b)
    res = run(in_maps)
    out = np.concatenate([res.results[c]["out"] for c in range(N_CORES)], axis=1)
    return np.ascontiguousarray(out.astype(np.float32, copy=False))
